# revision 43
# baseline (speedup 1.0000x reference)
"""FFT depthwise conv == direct 7x7 circular depthwise conv, on 8 TRN2 cores.

out[b,i,j,c] = sum_{u,v} wf[c,u,v] * x[b,(i+u-3)%H,(j+v-3)%W,c],  wf = kernel[:, ::-1, ::-1]

Sharding: data-parallel over batch (1 image per core). Host pre-pads each
image circularly to (C, 230, 230) so every on-device tile load is a single
contiguous-row DMA (no wrap handling on device).

Per core: partitions = 64 channels x 2 row-halves; 3 channel groups x 7
row-strips of 16 output rows.
  TensorE : N_PE_TAPS taps as diagonal-weight matmuls (fp32r), accumulated in
            PSUM per 2-row bank tile (7 rotating single-bank tiles)
  VectorE : remaining taps as fused MACs (scalar_tensor_tensor), then merges
            each PSUM bank into the accumulator in place; DMA reads the result
Engine instructions have tiny sync-wait budgets (GpSimd 1, VectorE ~2), so the
structure keeps every compute instruction at <=2 waits: "absorber" matmuls
soak up DMA-completion waits on PE, seeds are split so each carries only the
input-DMA + accumulator-WAR waits, and nosync ordering hints keep DVE seeds
after merges so transitive PE ticks are already observed.
"""

import os
import sys

for _p in ("/opt/trn_rl_repo", "/root/.axon_site/_ro/trn_rl_repo"):
    if os.path.isdir(_p) and _p not in sys.path:
        sys.path.insert(0, _p)

import numpy as np

import concourse.bacc as bacc
import concourse.bass as bass
import concourse.mybir as mybir
from concourse.bass_utils import run_bass_kernel_spmd
from concourse.tile import TileContext

F32 = mybir.dt.float32
F32R = mybir.dt.float32r
BF16 = mybir.dt.bfloat16

B, H, W, C, K = 8, 224, 224, 192, 7
NCORES = 8
PAD = K // 2          # 3
PH, PW = H + 2 * PAD, W + 2 * PAD  # 230, 230 padded image dims
HALF = H // 2         # 112 output rows per s-half
TH = 28               # output rows per strip (per half)
NSTRIP = HALF // TH   # 7
CG = 64               # channels per group
NG = C // CG          # 3
TROWS = TH + 2 * PAD  # 22 input rows per strip
TCOLS = PW            # 230 input cols per strip

# --- engine tap split (tunable) -------------------------------------------
# Odd-v taps are 2-byte-misaligned in the bf16 tile, which would knock the
# DVE out of its 2x perf mode -- so they are listed first and always land on
# the PE side of the split.
ALL_TAPS = sorted(
    ((u, v) for u in range(K) for v in range(K)),
    key=lambda t: (t[1] % 2 == 0, t[0], t[1]),
)
N_PE_TAPS = 34        # taps done on TensorE via diagonal matmuls (>= 21)
SUB = 14              # sub-strip rows (= 7 PSUM banks)
PE_TAPS = ALL_TAPS[:N_PE_TAPS]
VEC_TAPS = ALL_TAPS[N_PE_TAPS:]
USE_F32R = False
USE_BF16 = True

# DMA row-chunking: each chunk is one dma_start on its own queue/engine
IN_ROW_CHUNKS = [(0, 9), (9, 9), (18, 8), (26, 8)]     # covers TROWS=34
OUT_ROW_CHUNKS = [(0, 7), (7, 7)]                      # covers SUB=14


def _tap_idx(u, v):
    return u * K + v


def _add_dep(from_inst, to_inst):
    """Ordering-only (no-semaphore) dependency between two instructions."""
    import bass_rust as _br

    fi = getattr(from_inst, "ins", from_inst)
    ti = getattr(to_inst, "ins", to_inst)
    _br.add_dep_helper(fi, ti, sync=False, reason="seed-after-merge ordering")


def build_nc():
    # Bacc (not plain Bass): its compile() runs generate_event_semaphores,
    # which spills excess per-instruction sync waits onto EventSemaphore
    # instructions -- engine instructions only have 1 inline wait slot.
    nc = bacc.Bacc()
    xdt = BF16 if USE_BF16 else (F32R if USE_F32R else F32)
    odt = BF16 if USE_BF16 else F32
    x_d = nc.declare_dram_parameter("x", [C, PH, PW], xdt, isOutput=False)
    wvec_d = nc.declare_dram_parameter("wvec", [128, NG * K * K], F32, isOutput=False)
    wdiag_d = nc.declare_dram_parameter(
        "wdiag", [128, NG, K * K, 128], xdt, isOutput=False
    )
    out_d = nc.declare_dram_parameter("out", [C, H, W], odt, isOutput=True)

    mult = mybir.AluOpType.mult
    add = mybir.AluOpType.add
    act_copy = mybir.ActivationFunctionType.Copy

    with TileContext(nc) as tc:
        with (
            tc.tile_pool(name="consts", bufs=1) as cpool,
            tc.tile_pool(name="wdg", bufs=3) as wpool,
            tc.tile_pool(name="xin", bufs=4) as xpool,
            tc.tile_pool(name="xpre", bufs=2) as prepool,
            tc.tile_pool(name="accdp", bufs=3) as adpool,
            tc.tile_pool(name="tmpp", bufs=2) as tppool,
            tc.tile_pool(name="outp", bufs=3) as opool,
            tc.tile_pool(name="psum", bufs=7, space="PSUM") as ppool,
            tc.tile_pool(name="scr", bufs=1, space="PSUM") as spool,
        ):
            wvec_sb = cpool.tile([128, NG * K * K], F32)
            nc.sync.dma_start(out=wvec_sb[:], in_=wvec_d[:])
            scratch = spool.tile([128, 512], F32)

            prev_merge = [None]  # last DVE merge instruction of previous strip

            # preload ALL groups' diagonal weights up front so group
            # transitions never wait on a 1.6 MB weight DMA stuck behind
            # the queued input DMAs
            wdgs = []
            for g in range(NG):
                wdg = wpool.tile([128, K * K, 128], xdt, name=f"wdg{g}", tag="wdg")
                hkk = (K * K) // 2
                nc.sync.dma_start(out=wdg[:, 0:hkk, :], in_=wdiag_d[:, g, 0:hkk, :])
                nc.scalar.dma_start(
                    out=wdg[:, hkk:, :], in_=wdiag_d[:, g, hkk:, :]
                )
                wdgs.append(wdg)

            def issue_in_dma(dst_tile, g, t):
                xh = x_d.tensor if hasattr(x_d, "tensor") else x_d
                base = g * CG * PH * PW + t * TH * PW
                for ci, (ra, nr) in enumerate(IN_ROW_CHUNKS):
                    srcap = bass.AP(
                        xh,
                        base + ra * PW,
                        [[HALF * PW, 2], [PH * PW, CG], [PW, nr], [1, TCOLS]],
                    )
                    eng = nc.sync if ci % 2 == 0 else nc.scalar
                    eng.dma_start(out=dst_tile[:, ra:ra + nr, :], in_=srcap)

            pre_tiles = {}
            for g in range(NG):
                wdg = wdgs[g]
                # throwaway matmul absorbs the wdg-DMA wait on PE once per group
                nc.tensor.matmul(
                    scratch[:, 0:128], wdg[:, 0, :], wdg[:, 0, :],
                    start=True, stop=True,
                )
                # prefetch the NEXT group's first input tile now, so the
                # group transition never waits on a tile load queued behind
                # this whole group's DMA traffic
                if g + 1 < NG:
                    pre = prepool.tile(
                        [128, TROWS, TCOLS], xdt, name=f"xpre{g + 1}", tag="xpre"
                    )
                    issue_in_dma(pre, g + 1, 0)
                    pre_tiles[g + 1] = pre

                for t in range(NSTRIP):
                    if t == 0 and g in pre_tiles:
                        xt = pre_tiles.pop(g)
                    else:
                        xt = xpool.tile(
                            [128, TROWS, TCOLS], xdt, name=f"xt{g}_{t}", tag="xt"
                        )
                        issue_in_dma(xt, g, t)

                    # ---- PE absorber: first PE toucher of this xt tile ----
                    nc.tensor.matmul(
                        scratch[:, 0:128], wdg[:, 0, :], xt[:, 0, 0:128],
                        start=True, stop=True,
                    )

                    # ---- two 14-row sub-strips per DMA strip: each uses
                    # exactly 7 PSUM banks (the full rotation), so PE never
                    # stalls waiting for same-strip merges
                    for sub in range(TH // SUB):
                        sb = sub * SUB
                        acc = adpool.tile(
                            [128, SUB, W], BF16, name=f"acc{g}_{t}_{sub}", tag="acc"
                        )
                        outt = opool.tile(
                            [128, SUB, W], odt, name=f"outt{g}_{t}_{sub}", tag="outt"
                        )
                        tmps = [
                            tppool.tile(
                                [128, SUB, W], BF16,
                                name=f"tmp{g}_{t}_{sub}_{j}", tag=f"tmp{j}",
                            )
                            for j in range(2)
                        ]

                        # ---- vector taps on DVE: all-bf16 2-op MACs.
                        # tensor_scalar products run in 4x mode, tensor_tensor
                        # adds in 2x mode -- beats the 1x-capped fused STT.
                        u0, v0 = VEC_TAPS[0]
                        ti0 = g * K * K + _tap_idx(u0, v0)
                        wv0 = wvec_sb[:, ti0:ti0 + 1]
                        seed = nc.vector.tensor_scalar(
                            acc[:],
                            xt[:, u0 + sb:u0 + sb + SUB, v0:v0 + W],
                            wv0,
                            None,
                            mult,
                        )
                        if prev_merge[0] is not None:
                            _add_dep(seed, prev_merge[0])
                        for j, (u, v) in enumerate(VEC_TAPS[1:]):
                            ti = g * K * K + _tap_idx(u, v)
                            wv = wvec_sb[:, ti:ti + 1]
                            tmp = tmps[j % 2]
                            nc.vector.tensor_scalar(
                                tmp[:],
                                xt[:, u + sb:u + sb + SUB, v:v + W],
                                wv,
                                None,
                                mult,
                            )
                            nc.vector.tensor_tensor(acc[:], acc[:], tmp[:], add)

                        # ---- TensorE taps: SUB/2 bank-tiles of 2 rows ----
                        n_pe = len(PE_TAPS)
                        for b8 in range(SUB // 2):
                            ps = ppool.tile(
                                [128, 512], F32, name=f"ps{g}_{t}_{sub}_{b8}", tag="ps"
                            )
                            row0 = 2 * b8
                            for ti, (u, v) in enumerate(PE_TAPS):
                                rhs = xt[:, u + sb + row0:u + sb + row0 + 2, v:v + W]
                                nc.tensor.matmul(
                                    ps[:, 0:2 * W],
                                    wdg[:, _tap_idx(u, v), :],
                                    rhs,
                                    start=(ti == 0),
                                    stop=(ti == n_pe - 1),
                                )
                            # merge psum + acc -> bf16 output tile (DVE)
                            ps3 = ps[:, 0:2 * W].rearrange("p (r w) -> p r w", r=2)
                            mg = nc.vector.scalar_tensor_tensor(
                                outt[:, row0:row0 + 2, :],
                                ps3,
                                1.0,
                                acc[:, row0:row0 + 2, :],
                                mult,
                                add,
                            )
                        prev_merge[0] = mg

                        # ---- output DMA per sub-strip, row-chunked ----
                        oh = out_d.tensor if hasattr(out_d, "tensor") else out_d
                        obase = g * CG * H * W + (t * TH + sb) * W
                        for ci, (ra, nr) in enumerate(OUT_ROW_CHUNKS):
                            dst = bass.AP(
                                oh,
                                obase + ra * W,
                                [[HALF * W, 2], [H * W, CG], [W, nr], [1, W]],
                            )
                            eng = nc.scalar if ci % 2 == 0 else nc.sync
                            eng.dma_start(out=dst, in_=outt[:, ra:ra + nr, :])
    return nc


def _host_weights(kernel):
    """kernel: (C, K, K) -> (wvec [128, NG*49], wdiag [128, NG, 49, 128])."""
    wf = kernel[:, ::-1, ::-1].astype(np.float32)  # flipped: cross-correlation form
    cl = np.arange(128) % CG  # channel-local index per partition
    wvec = np.empty((128, NG * K * K), dtype=np.float32)
    wdiag = np.zeros((128, NG, K * K, 128), dtype=np.float32)
    eye = np.arange(128)
    for g in range(NG):
        wg = wf[g * CG:(g + 1) * CG].reshape(CG, K * K)  # (64, 49)
        wvec[:, g * K * K:(g + 1) * K * K] = wg[cl]
        wdiag[eye, g, :, eye] = wg[cl]
    return wvec, wdiag


_NC_CACHE = {}


def _get_nc():
    if "nc" not in _NC_CACHE:
        nc = build_nc()
        # Bacc passes (register alloc, EventSemaphore wait-splitting, ...)
        # run in finalize(); the pjrt path serializes the module as-is, so
        # finalize here before handing it off.
        nc.finalize()
        _NC_CACHE["nc"] = nc
    return _NC_CACHE["nc"]


def run(x, kernel, trace=False, **kw):
    assert x.shape == (B, H, W, C) and kernel.shape == (C, K, K)
    nc = _get_nc()
    xT = np.ascontiguousarray(x.transpose(0, 3, 1, 2)).astype(np.float32)  # (B,C,H,W)
    xTp = np.pad(xT, ((0, 0), (0, 0), (PAD, PAD), (PAD, PAD)), mode="wrap")
    xTp = np.ascontiguousarray(xTp)
    wvec, wdiag = _host_weights(np.asarray(kernel))
    if USE_BF16:
        import ml_dtypes

        xTp = xTp.astype(ml_dtypes.bfloat16)
        wdiag = wdiag.astype(ml_dtypes.bfloat16)
    in_maps = [{"x": xTp[b], "wvec": wvec, "wdiag": wdiag} for b in range(NCORES)]
    res = run_bass_kernel_spmd(nc, in_maps, list(range(NCORES)), trace=trace, **kw)
    out = np.stack(
        [np.asarray(res.results[b]["out"]).astype(np.float32) for b in range(NCORES)]
    )
    out = np.ascontiguousarray(out.transpose(0, 2, 3, 1)).astype(np.float32)
    return out, res


def kernel(x, kernel):
    out, _ = run(np.asarray(x), np.asarray(kernel))
    return out


# revision 44
# speedup vs baseline: 1.0054x; 1.0054x over previous
"""FFT depthwise conv == direct 7x7 circular depthwise conv, on 8 TRN2 cores.

out[b,i,j,c] = sum_{u,v} wf[c,u,v] * x[b,(i+u-3)%H,(j+v-3)%W,c],  wf = kernel[:, ::-1, ::-1]

Sharding: data-parallel over batch (1 image per core). Host pre-pads each
image circularly to (C, 230, 230) so every on-device tile load is a single
contiguous-row DMA (no wrap handling on device).

Per core: partitions = 64 channels x 2 row-halves; 3 channel groups x 7
row-strips of 16 output rows.
  TensorE : N_PE_TAPS taps as diagonal-weight matmuls (fp32r), accumulated in
            PSUM per 2-row bank tile (7 rotating single-bank tiles)
  VectorE : remaining taps as fused MACs (scalar_tensor_tensor), then merges
            each PSUM bank into the accumulator in place; DMA reads the result
Engine instructions have tiny sync-wait budgets (GpSimd 1, VectorE ~2), so the
structure keeps every compute instruction at <=2 waits: "absorber" matmuls
soak up DMA-completion waits on PE, seeds are split so each carries only the
input-DMA + accumulator-WAR waits, and nosync ordering hints keep DVE seeds
after merges so transitive PE ticks are already observed.
"""

import os
import sys

for _p in ("/opt/trn_rl_repo", "/root/.axon_site/_ro/trn_rl_repo"):
    if os.path.isdir(_p) and _p not in sys.path:
        sys.path.insert(0, _p)

import numpy as np

import concourse.bacc as bacc
import concourse.bass as bass
import concourse.mybir as mybir
from concourse.bass_utils import run_bass_kernel_spmd
from concourse.tile import TileContext

F32 = mybir.dt.float32
F32R = mybir.dt.float32r
BF16 = mybir.dt.bfloat16

B, H, W, C, K = 8, 224, 224, 192, 7
NCORES = 8
PAD = K // 2          # 3
PH, PW = H + 2 * PAD, W + 2 * PAD  # 230, 230 padded image dims
HALF = H // 2         # 112 output rows per s-half
TH = 28               # output rows per strip (per half)
NSTRIP = HALF // TH   # 7
CG = 64               # channels per group
NG = C // CG          # 3
TROWS = TH + 2 * PAD  # 22 input rows per strip
TCOLS = PW            # 230 input cols per strip

# --- engine tap split (tunable) -------------------------------------------
# Odd-v taps are 2-byte-misaligned in the bf16 tile, which would knock the
# DVE out of its 2x perf mode -- so they are listed first and always land on
# the PE side of the split.
ALL_TAPS = sorted(
    ((u, v) for u in range(K) for v in range(K)),
    key=lambda t: (t[1] % 2 == 0, t[0], t[1]),
)
N_PE_TAPS = 34        # taps done on TensorE via diagonal matmuls (>= 21)
SUB = 14              # sub-strip rows (= 7 PSUM banks)
PE_TAPS = ALL_TAPS[:N_PE_TAPS]
VEC_TAPS = ALL_TAPS[N_PE_TAPS:]
USE_F32R = False
USE_BF16 = True

# DMA row-chunking: each chunk is one dma_start on its own queue/engine
IN_ROW_CHUNKS = [(0, 9), (9, 9), (18, 8), (26, 8)]     # covers TROWS=34
OUT_ROW_CHUNKS = [(0, 7), (7, 7)]                      # covers SUB=14


def _tap_idx(u, v):
    return u * K + v


def _add_dep(from_inst, to_inst):
    """Ordering-only (no-semaphore) dependency between two instructions."""
    import bass_rust as _br

    fi = getattr(from_inst, "ins", from_inst)
    ti = getattr(to_inst, "ins", to_inst)
    _br.add_dep_helper(fi, ti, sync=False, reason="seed-after-merge ordering")


def build_nc():
    # Bacc (not plain Bass): its compile() runs generate_event_semaphores,
    # which spills excess per-instruction sync waits onto EventSemaphore
    # instructions -- engine instructions only have 1 inline wait slot.
    nc = bacc.Bacc()
    xdt = BF16 if USE_BF16 else (F32R if USE_F32R else F32)
    odt = BF16 if USE_BF16 else F32
    x_d = nc.declare_dram_parameter("x", [C, PH, PW], xdt, isOutput=False)
    wvec_d = nc.declare_dram_parameter("wvec", [128, NG * K * K], F32, isOutput=False)
    wdiag_d = nc.declare_dram_parameter(
        "wdiag", [128, NG, K * K, 128], xdt, isOutput=False
    )
    out_d = nc.declare_dram_parameter("out", [C, H, W], odt, isOutput=True)

    mult = mybir.AluOpType.mult
    add = mybir.AluOpType.add
    act_copy = mybir.ActivationFunctionType.Copy

    with TileContext(nc) as tc:
        with (
            tc.tile_pool(name="consts", bufs=1) as cpool,
            tc.tile_pool(name="wdg", bufs=3) as wpool,
            tc.tile_pool(name="xin", bufs=4) as xpool,
            tc.tile_pool(name="xpre", bufs=2) as prepool,
            tc.tile_pool(name="accdp", bufs=3) as adpool,
            tc.tile_pool(name="tmpp", bufs=2) as tppool,
            tc.tile_pool(name="outp", bufs=3) as opool,
            tc.tile_pool(name="psum", bufs=7, space="PSUM") as ppool,
            tc.tile_pool(name="scr", bufs=1, space="PSUM") as spool,
        ):
            wvec_sb = cpool.tile([128, NG * K * K], F32)
            nc.sync.dma_start(out=wvec_sb[:], in_=wvec_d[:])
            scratch = spool.tile([128, 512], F32)

            prev_merge = [None]  # last DVE merge instruction of previous strip

            # preload ALL groups' diagonal weights up front so group
            # transitions never wait on a 1.6 MB weight DMA stuck behind
            # the queued input DMAs
            wdgs = []
            for g in range(NG):
                wdg = wpool.tile([128, K * K, 128], xdt, name=f"wdg{g}", tag="wdg")
                hkk = (K * K) // 2
                nc.sync.dma_start(out=wdg[:, 0:hkk, :], in_=wdiag_d[:, g, 0:hkk, :])
                nc.scalar.dma_start(
                    out=wdg[:, hkk:, :], in_=wdiag_d[:, g, hkk:, :]
                )
                wdgs.append(wdg)

            def issue_in_dma(dst_tile, g, t):
                xh = x_d.tensor if hasattr(x_d, "tensor") else x_d
                base = g * CG * PH * PW + t * TH * PW
                for ci, (ra, nr) in enumerate(IN_ROW_CHUNKS):
                    srcap = bass.AP(
                        xh,
                        base + ra * PW,
                        [[HALF * PW, 2], [PH * PW, CG], [PW, nr], [1, TCOLS]],
                    )
                    eng = nc.sync if ci % 2 == 0 else nc.scalar
                    eng.dma_start(out=dst_tile[:, ra:ra + nr, :], in_=srcap)

            pre_tiles = {}
            for g in range(NG):
                wdg = wdgs[g]
                # throwaway matmul absorbs the wdg-DMA wait on PE once per group
                nc.tensor.matmul(
                    scratch[:, 0:128], wdg[:, 0, :], wdg[:, 0, :],
                    start=True, stop=True,
                )
                # prefetch the NEXT group's first input tile now, so the
                # group transition never waits on a tile load queued behind
                # this whole group's DMA traffic
                if g + 1 < NG:
                    pre = prepool.tile(
                        [128, TROWS, TCOLS], xdt, name=f"xpre{g + 1}", tag="xpre"
                    )
                    issue_in_dma(pre, g + 1, 0)
                    pre_tiles[g + 1] = pre

                for t in range(NSTRIP):
                    if t == 0 and g in pre_tiles:
                        xt = pre_tiles.pop(g)
                    else:
                        xt = xpool.tile(
                            [128, TROWS, TCOLS], xdt, name=f"xt{g}_{t}", tag="xt"
                        )
                        issue_in_dma(xt, g, t)

                    # ---- PE absorber: first PE toucher of this xt tile ----
                    nc.tensor.matmul(
                        scratch[:, 0:128], wdg[:, 0, :], xt[:, 0, 0:128],
                        start=True, stop=True,
                    )

                    # ---- two 14-row sub-strips per DMA strip: each uses
                    # exactly 7 PSUM banks (the full rotation), so PE never
                    # stalls waiting for same-strip merges
                    for sub in range(TH // SUB):
                        sb = sub * SUB
                        acc = adpool.tile(
                            [128, SUB, W], BF16, name=f"acc{g}_{t}_{sub}", tag="acc"
                        )
                        outt = opool.tile(
                            [128, SUB, W], odt, name=f"outt{g}_{t}_{sub}", tag="outt"
                        )
                        tmps = [
                            tppool.tile(
                                [128, SUB, W], BF16,
                                name=f"tmp{g}_{t}_{sub}_{j}", tag=f"tmp{j}",
                            )
                            for j in range(2)
                        ]

                        # ---- vector taps on DVE: all-bf16 2-op MACs.
                        # tensor_scalar products run in 4x mode, tensor_tensor
                        # adds in 2x mode -- beats the 1x-capped fused STT.
                        u0, v0 = VEC_TAPS[0]
                        ti0 = g * K * K + _tap_idx(u0, v0)
                        wv0 = wvec_sb[:, ti0:ti0 + 1]
                        seed = nc.vector.tensor_scalar(
                            acc[:],
                            xt[:, u0 + sb:u0 + sb + SUB, v0:v0 + W],
                            wv0,
                            None,
                            mult,
                        )
                        if prev_merge[0] is not None:
                            _add_dep(seed, prev_merge[0])
                        for j, (u, v) in enumerate(VEC_TAPS[1:]):
                            ti = g * K * K + _tap_idx(u, v)
                            wv = wvec_sb[:, ti:ti + 1]
                            tmp = tmps[j % 2]
                            nc.vector.tensor_scalar(
                                tmp[:],
                                xt[:, u + sb:u + sb + SUB, v:v + W],
                                wv,
                                None,
                                mult,
                            )
                            nc.vector.tensor_tensor(acc[:], acc[:], tmp[:], add)

                        # ---- TensorE taps: SUB/2 bank-tiles of 2 rows ----
                        n_pe = len(PE_TAPS)
                        for b8 in range(SUB // 2):
                            ps = ppool.tile(
                                [128, 512], F32, name=f"ps{g}_{t}_{sub}_{b8}", tag="ps"
                            )
                            row0 = 2 * b8
                            for ti, (u, v) in enumerate(PE_TAPS):
                                rhs = xt[:, u + sb + row0:u + sb + row0 + 2, v:v + W]
                                nc.tensor.matmul(
                                    ps[:, 0:2 * W],
                                    wdg[:, _tap_idx(u, v), :],
                                    rhs,
                                    start=(ti == 0),
                                    stop=(ti == n_pe - 1),
                                )
                            # merge psum + acc -> bf16 output tile (DVE)
                            ps3 = ps[:, 0:2 * W].rearrange("p (r w) -> p r w", r=2)
                            mg = nc.vector.scalar_tensor_tensor(
                                outt[:, row0:row0 + 2, :],
                                ps3,
                                1.0,
                                acc[:, row0:row0 + 2, :],
                                mult,
                                add,
                            )
                            if b8 == 0:
                                # the ordering hint for the next seed points at
                                # the FIRST merge: enough to cover transitive
                                # PE ticks, without serializing the next
                                # sub-strip behind PE's last bank
                                prev_merge[0] = mg

                        # ---- output DMA per sub-strip, row-chunked ----
                        oh = out_d.tensor if hasattr(out_d, "tensor") else out_d
                        obase = g * CG * H * W + (t * TH + sb) * W
                        for ci, (ra, nr) in enumerate(OUT_ROW_CHUNKS):
                            dst = bass.AP(
                                oh,
                                obase + ra * W,
                                [[HALF * W, 2], [H * W, CG], [W, nr], [1, W]],
                            )
                            eng = nc.scalar if ci % 2 == 0 else nc.sync
                            eng.dma_start(out=dst, in_=outt[:, ra:ra + nr, :])
    return nc


def _host_weights(kernel):
    """kernel: (C, K, K) -> (wvec [128, NG*49], wdiag [128, NG, 49, 128])."""
    wf = kernel[:, ::-1, ::-1].astype(np.float32)  # flipped: cross-correlation form
    cl = np.arange(128) % CG  # channel-local index per partition
    wvec = np.empty((128, NG * K * K), dtype=np.float32)
    wdiag = np.zeros((128, NG, K * K, 128), dtype=np.float32)
    eye = np.arange(128)
    for g in range(NG):
        wg = wf[g * CG:(g + 1) * CG].reshape(CG, K * K)  # (64, 49)
        wvec[:, g * K * K:(g + 1) * K * K] = wg[cl]
        wdiag[eye, g, :, eye] = wg[cl]
    return wvec, wdiag


_NC_CACHE = {}


def _get_nc():
    if "nc" not in _NC_CACHE:
        nc = build_nc()
        # Bacc passes (register alloc, EventSemaphore wait-splitting, ...)
        # run in finalize(); the pjrt path serializes the module as-is, so
        # finalize here before handing it off.
        nc.finalize()
        _NC_CACHE["nc"] = nc
    return _NC_CACHE["nc"]


def run(x, kernel, trace=False, **kw):
    assert x.shape == (B, H, W, C) and kernel.shape == (C, K, K)
    nc = _get_nc()
    xT = np.ascontiguousarray(x.transpose(0, 3, 1, 2)).astype(np.float32)  # (B,C,H,W)
    xTp = np.pad(xT, ((0, 0), (0, 0), (PAD, PAD), (PAD, PAD)), mode="wrap")
    xTp = np.ascontiguousarray(xTp)
    wvec, wdiag = _host_weights(np.asarray(kernel))
    if USE_BF16:
        import ml_dtypes

        xTp = xTp.astype(ml_dtypes.bfloat16)
        wdiag = wdiag.astype(ml_dtypes.bfloat16)
    in_maps = [{"x": xTp[b], "wvec": wvec, "wdiag": wdiag} for b in range(NCORES)]
    res = run_bass_kernel_spmd(nc, in_maps, list(range(NCORES)), trace=trace, **kw)
    out = np.stack(
        [np.asarray(res.results[b]["out"]).astype(np.float32) for b in range(NCORES)]
    )
    out = np.ascontiguousarray(out.transpose(0, 2, 3, 1)).astype(np.float32)
    return out, res


def kernel(x, kernel):
    out, _ = run(np.asarray(x), np.asarray(kernel))
    return out
